# revision 1
# baseline (speedup 1.0000x reference)
"""GCN classifier on 8 TRN2 NeuronCores.

Math (reference):
    h1  = relu(adj @ (X @ W1) + b1)        [N, D]
    h2  = relu(adj @ (h1 @ W2) + b2)       [N, D]
    h3  = relu(h2 @ Wm1 + bm1)             [N, D]
    out = h3 @ Wm2 + bm2                   [N, 1]

Sharding: 1D row partition of adj over 8 cores (2048 rows each). Each core
receives its shard PRE-TRANSPOSED on the host (B_c = adj[rows_c, :].T,
shape [N, P]) so that every on-device matmul contracts over the SBUF
partition axis with operands in natural layout:

    layer 1:  Z1.T = X.T @ B_c            (lhsT = X k-block tiles [128, 64])
              h1.T = relu(W1.T @ Z1.T + b1)
    gather:   AllGather h1.T shards -> full h1.T on every core
    layer 2:  G[kb] = (h1.T slice).T @ W2  (tiny matmul == free transpose)
              Z2.T = G.T @ B_c, h2.T = relu(Z2.T + b2)
    head:     h3.T = relu(Wm1.T @ h2.T + bm1); out.T = Wm2.T @ h3.T + bm2

HBM traffic per core is dominated by streaming B_c twice (2 x 128 MiB).
"""

import numpy as np

N = 16384
D = 64
N_CORES = 8
P = N // N_CORES          # 2048 nodes per core
KB = N // 128             # 128 contraction blocks of 128
T_PER_CORE = P // 128     # 16 local row-blocks per core
IC = 512                  # matmul moving free-dim chunk
NIC = P // IC             # 4 chunks per slab
SLAB_KB = 2               # k-blocks per DMA slab (2 MiB per dma_start)

# fp32 matmuls run at 4 cycles/row on the PE; float32r runs at 1 cycle/row
# for moving dims >= 256 (same bits in SBUF, reduced-precision multiply).
MM_DTYPE_NAME = "float32"

_cache = {}


def _build(mm_dtype_name=MM_DTYPE_NAME):
    import concourse.bass as bass  # noqa: F401  (registers engines)
    import concourse.mybir as mybir
    import concourse.tile as tile
    from concourse import bacc

    f32 = mybir.dt.float32
    mmdt = getattr(mybir.dt, mm_dtype_name)

    nc = bacc.Bacc("TRN2", target_bir_lowering=False, debug=False,
                   num_devices=N_CORES)

    adjT = nc.dram_tensor("adjT", [N, P], mmdt, kind="ExternalInput")
    xb = nc.dram_tensor("xb", [128, KB, D], mmdt, kind="ExternalInput")
    w1 = nc.dram_tensor("w1", [D, D], mmdt, kind="ExternalInput")
    b1 = nc.dram_tensor("b1", [D, 1], f32, kind="ExternalInput")
    w2 = nc.dram_tensor("w2", [D, D], mmdt, kind="ExternalInput")
    b2 = nc.dram_tensor("b2", [D, 1], f32, kind="ExternalInput")
    wm1 = nc.dram_tensor("wm1", [D, D], mmdt, kind="ExternalInput")
    bm1 = nc.dram_tensor("bm1", [D, 1], f32, kind="ExternalInput")
    wm2 = nc.dram_tensor("wm2", [D, 1], mmdt, kind="ExternalInput")
    bm2 = nc.dram_tensor("bm2", [1, 1], f32, kind="ExternalInput")
    out = nc.dram_tensor("out", [1, P], f32, kind="ExternalOutput")

    # collective bounce buffers (internal DRAM)
    hb_in = nc.dram_tensor("hb_in", [D, P], mmdt)
    hb_out = nc.dram_tensor("hb_out", [N_CORES * D, P], mmdt)

    n_slabs = KB // SLAB_KB
    relu = mybir.ActivationFunctionType.Relu
    ident = mybir.ActivationFunctionType.Identity

    with tile.TileContext(nc) as tc:
        with (
            tc.tile_pool(name="bpool", bufs=3) as bpool,
            tc.tile_pool(name="big", bufs=1) as big,
            tc.tile_pool(name="wpool", bufs=1) as wpool,
            tc.tile_pool(name="hv", bufs=2) as hv,
            tc.tile_pool(name="gpool", bufs=3) as gpool,
            tc.tile_pool(name="opool", bufs=1) as opool,
            tc.tile_pool(name="pacc", bufs=1, space="PSUM") as pacc,
            tc.tile_pool(name="psmall", bufs=2, space="PSUM") as psmall,
        ):
            # ---- constants ----
            xfull = big.tile([128, KB, D], mmdt, tag="big")
            nc.sync.dma_start(xfull[:], xb[:])
            w1t = wpool.tile([D, D], mmdt, tag="w1")
            nc.gpsimd.dma_start(w1t[:], w1[:])
            w2t = wpool.tile([D, D], mmdt, tag="w2")
            nc.gpsimd.dma_start(w2t[:], w2[:])
            wm1t = wpool.tile([D, D], mmdt, tag="wm1")
            nc.gpsimd.dma_start(wm1t[:], wm1[:])
            wm2t = wpool.tile([D, 1], mmdt, tag="wm2")
            nc.gpsimd.dma_start(wm2t[:], wm2[:])
            b1t = wpool.tile([D, 1], f32, tag="b1")
            nc.gpsimd.dma_start(b1t[:], b1[:])
            b2t = wpool.tile([D, 1], f32, tag="b2")
            nc.gpsimd.dma_start(b2t[:], b2[:])
            bm1t = wpool.tile([D, 1], f32, tag="bm1")
            nc.gpsimd.dma_start(bm1t[:], bm1[:])
            bm2t = wpool.tile([1, 1], f32, tag="bm2")
            nc.gpsimd.dma_start(bm2t[:], bm2[:])

            # ---- layer 1: Z1.T = X.T @ B  (accumulate over all 128 kb) ----
            pz = pacc.tile([D, P], f32, tag="acc")
            for s in range(n_slabs):
                slab = bpool.tile([128, SLAB_KB, P], mmdt, tag="slab")
                src = adjT[s * SLAB_KB * 128:(s + 1) * SLAB_KB * 128, :]
                nc.sync.dma_start(slab[:], src.rearrange("(n p) i -> p n i", p=128))
                for j in range(SLAB_KB):
                    kb = s * SLAB_KB + j
                    for ic in range(NIC):
                        nc.tensor.matmul(
                            pz[:, ic * IC:(ic + 1) * IC],
                            xfull[:, kb, :],
                            slab[:, j, ic * IC:(ic + 1) * IC],
                            start=(kb == 0),
                            stop=(kb == KB - 1),
                        )

            # ---- h1.T = relu(W1.T @ Z1.T + b1) ----
            z1 = hv.tile([D, P], mmdt, tag="hv")
            nc.vector.tensor_copy(z1[:], pz[:])
            ph = pacc.tile([D, P], f32, tag="acc")
            for ic in range(NIC):
                nc.tensor.matmul(ph[:, ic * IC:(ic + 1) * IC], w1t[:],
                                 z1[:, ic * IC:(ic + 1) * IC],
                                 start=True, stop=True)
            h1 = hv.tile([D, P], mmdt, tag="hv")
            nc.scalar.activation(h1[:], ph[:], relu, bias=b1t[:])

            # ---- AllGather h1.T ----
            nc.sync.dma_start(hb_in[:], h1[:])
            nc.gpsimd.collective_compute(
                "AllGather",
                mybir.AluOpType.bypass,
                replica_groups=[list(range(N_CORES))],
                ins=[hb_in.ap().opt()],
                outs=[hb_out.ap().opt()],
            )
            hfull = big.tile([D, N_CORES, P], mmdt, tag="big")
            nc.sync.dma_start(hfull[:],
                              hb_out.rearrange("(c j) i -> j c i", j=D))

            # ---- layer 2: G[kb] = (h1.T slice).T @ W2 ; Z2.T = G.T @ B ----
            pz2 = pacc.tile([D, P], f32, tag="acc")
            for s in range(n_slabs):
                slab = bpool.tile([128, SLAB_KB, P], mmdt, tag="slab")
                src = adjT[s * SLAB_KB * 128:(s + 1) * SLAB_KB * 128, :]
                nc.sync.dma_start(slab[:], src.rearrange("(n p) i -> p n i", p=128))
                for j in range(SLAB_KB):
                    kb = s * SLAB_KB + j
                    c_, t_ = divmod(kb, T_PER_CORE)
                    pg = psmall.tile([128, D], f32, tag="spg")
                    nc.tensor.matmul(pg[:],
                                     hfull[:, c_, t_ * 128:(t_ + 1) * 128],
                                     w2t[:], start=True, stop=True)
                    g = gpool.tile([128, D], mmdt, tag="g")
                    nc.vector.tensor_copy(g[:], pg[:])
                    for ic in range(NIC):
                        nc.tensor.matmul(
                            pz2[:, ic * IC:(ic + 1) * IC],
                            g[:],
                            slab[:, j, ic * IC:(ic + 1) * IC],
                            start=(kb == 0),
                            stop=(kb == KB - 1),
                        )

            # ---- h2.T = relu(Z2.T + b2) ----
            h2 = hv.tile([D, P], mmdt, tag="hv")
            nc.scalar.activation(h2[:], pz2[:], relu, bias=b2t[:])

            # ---- head ----
            p3 = pacc.tile([D, P], f32, tag="acc")
            for ic in range(NIC):
                nc.tensor.matmul(p3[:, ic * IC:(ic + 1) * IC], wm1t[:],
                                 h2[:, ic * IC:(ic + 1) * IC],
                                 start=True, stop=True)
            h3 = hv.tile([D, P], mmdt, tag="hv")
            nc.scalar.activation(h3[:], p3[:], relu, bias=bm1t[:])

            outsb = opool.tile([1, P], f32, tag="out")
            for ic in range(NIC):
                po = psmall.tile([1, IC], f32, tag="spg")
                nc.tensor.matmul(po[:], wm2t[:],
                                 h3[:, ic * IC:(ic + 1) * IC],
                                 start=True, stop=True)
                nc.scalar.activation(outsb[:, ic * IC:(ic + 1) * IC], po[:],
                                     ident, bias=bm2t[:])
            nc.sync.dma_start(out[:], outsb[:])

    nc.compile()
    return nc


def _shard_adj(adj):
    """Per-core transposed shards B_c = adj[rows_c, :].T, contiguous."""
    shards = []
    for c in range(N_CORES):
        block = adj[c * P:(c + 1) * P, :]              # [P, N]
        bt = np.empty((N, P), dtype=np.float32)
        # blocked transpose: column-chunk of the source at a time
        step = 1024
        for k0 in range(0, N, step):
            bt[k0:k0 + step, :] = block[:, k0:k0 + step].T
        shards.append(bt)
    return shards


def _prep_inputs(adj, features, W1, b1, W2, b2, Wm1, bm1, Wm2, bm2):
    adj = np.ascontiguousarray(adj, dtype=np.float32)
    x = np.ascontiguousarray(features, dtype=np.float32)
    # xb[p, kb, d] = X[kb*128 + p, d]
    xb = np.ascontiguousarray(x.reshape(KB, 128, D).transpose(1, 0, 2))
    shards = _shard_adj(adj)
    common = {
        "xb": xb,
        "w1": np.ascontiguousarray(W1, dtype=np.float32),
        "b1": np.ascontiguousarray(b1, dtype=np.float32).reshape(D, 1),
        "w2": np.ascontiguousarray(W2, dtype=np.float32),
        "b2": np.ascontiguousarray(b2, dtype=np.float32).reshape(D, 1),
        "wm1": np.ascontiguousarray(Wm1, dtype=np.float32),
        "bm1": np.ascontiguousarray(bm1, dtype=np.float32).reshape(D, 1),
        "wm2": np.ascontiguousarray(Wm2, dtype=np.float32).reshape(D, 1),
        "bm2": np.ascontiguousarray(bm2, dtype=np.float32).reshape(1, 1),
    }
    return [dict(common, adjT=shards[c]) for c in range(N_CORES)]


def _run(in_maps, trace=False, **kw):
    from concourse.bass_utils import run_bass_kernel_spmd

    if "nc" not in _cache:
        _cache["nc"] = _build()
    res = run_bass_kernel_spmd(_cache["nc"], in_maps,
                               core_ids=list(range(N_CORES)),
                               trace=trace, **kw)
    full = np.concatenate([r["out"][0] for r in res.results])[:, None]
    return full.astype(np.float32), res


def kernel(adj, features, W1, b1, W2, b2, Wm1, bm1, Wm2, bm2):
    in_maps = _prep_inputs(adj, features, W1, b1, W2, b2, Wm1, bm1, Wm2, bm2)
    out, _ = _run(in_maps)
    return out
